# revision 5
# baseline (speedup 1.0000x reference)
"""CLIP contrastive loss on 8 Trainium2 NeuronCores (Bass/Tile).

Strategy (data-parallel over image rows, hint's local_loss path):
  - Core c holds image rows [c*1024, (c+1)*1024) and the FULL text matrix.
  - Text rows are rolled by c*1024 on the host so every core's diagonal
    block sits at local column 0 (the compiled program is core-independent).
  - Features are scaled by 16 and quantized to fp8e4 (e4m3) on the host;
    matmuls run in DoubleRow perf mode (two K=128 chunks per instruction,
    2x PE throughput vs bf16), accumulating f32 in PSUM. PSUM holds
    scale^2 * cos, folded back via the activation's scale operand.
  - Each core computes its 1024 x 8192 logits block in 128x2048 PSUM
    tiles (4 banks, double buffered), then:
      * ACT: one exp(act_scale*s - shift) per wide tile PSUM->SBUF (bf16),
        accum_out = per-row sums (free with the exp pass)
      * DVE: adds exp tiles into a per-nb [128,2048] bf16 column
        accumulator (DMA'd out whole); per-mt diagonal extracted with
        tensor_mul against act_scale*I + reduce
  - Host: partition-reduces the column accumulators and combines
    per-core row/col exp-sums and diagonals in float64:
      lse = shift + log(sum); loss = mean over both directions.

Fixed-shift logsumexp is numerically safe: logits = scale*cos(theta) are
bounded by +-scale, and shift = scale/2 keeps every term that matters in
normal f32 range (terms below exp(-87) are negligible vs the row max).
fp8 quantization of unit-norm features adds ~2e-3 absolute noise per
cosine; the resulting loss error is ~5e-4 relative (measured on host).
"""

from contextlib import ExitStack

import ml_dtypes
import numpy as np

import concourse.bass as bass
from concourse import bacc
import concourse.tile as tile
from concourse import mybir
from concourse.bass import ts
from concourse.bass_utils import run_bass_kernel_spmd

N = 8192
D = 512
NC = 8
M_LOC = N // NC          # 1024 image rows per core
MT = M_LOC // 128        # 8 m-tiles of 128 rows
KC = D // 128            # 4 contraction chunks of 128
KP = KC // 2             # 2 DoubleRow pairs
W = 2048                 # columns per PSUM tile (4 banks)
NB = N // W              # 4 col blocks
MM_W = 512               # columns per matmul instruction (ISA caps moving elems)
NH = W // MM_W

FEAT_SCALE = 16.0        # fp8 quantization scale (folded out in activation)

F32 = mybir.dt.float32
BF16 = mybir.dt.bfloat16
FP8 = mybir.dt.float8e4
NP_FP8 = ml_dtypes.float8_e4m3

MM_DTYPE = "fp8"         # informational only

_CACHE = {}
LAST_RESULTS = None


def _build(scale: float, shift: float):
    act_scale = scale / (FEAT_SCALE * FEAT_SCALE)
    nc = bacc.Bacc("TRN2", debug=False)

    at_d = nc.dram_tensor("at_in", [128, KC, M_LOC], FP8, kind="ExternalInput").ap()
    bt_d = nc.dram_tensor("bt_in", [NB, 128, KC, W], FP8, kind="ExternalInput").ap()
    eye_d = nc.dram_tensor("eye_in", [128, 128], F32, kind="ExternalInput").ap()

    rowsum_d = nc.dram_tensor("rowsum_out", [128, MT], F32, kind="ExternalOutput").ap()
    colsum_d = nc.dram_tensor("colsum_out", [NB, 128, W], BF16, kind="ExternalOutput").ap()
    diag_d = nc.dram_tensor("diag_out", [128, MT], F32, kind="ExternalOutput").ap()

    with ExitStack() as ctx:
        tc = ctx.enter_context(tile.TileContext(nc))
        singles = ctx.enter_context(tc.tile_pool(name="singles", bufs=1))
        btp = ctx.enter_context(tc.tile_pool(name="btp", bufs=NB))
        expp = ctx.enter_context(tc.tile_pool(name="expp", bufs=6))
        scr = ctx.enter_context(tc.tile_pool(name="scr", bufs=2))
        caccp = ctx.enter_context(tc.tile_pool(name="caccp", bufs=2))
        psum = ctx.enter_context(tc.tile_pool(name="psum", bufs=2, space="PSUM"))

        at_t = singles.tile([128, KC, M_LOC], FP8)
        bt_tiles = [
            btp.tile([128, KC, W], FP8, name=f"bt{nb}", tag="bt") for nb in range(NB)
        ]
        # Spread input loads over three DGE queues (SP + ACT hardware DGE,
        # Pool software DGE): a single queue tops out ~95 GB/s (descriptor
        # limited), and the first (at, bt0) MB gates the whole pipeline.
        nc.sync.dma_start(at_t, at_d)
        nc.scalar.dma_start(bt_tiles[0][:, 0:2, :], bt_d[0, :, 0:2, :])
        nc.gpsimd.dma_start(bt_tiles[0][:, 2:4, :], bt_d[0, :, 2:4, :])
        eye_t = singles.tile([128, 128], F32)
        nc.sync.dma_start(eye_t, eye_d)
        bias_t = singles.tile([128, 1], F32)
        nc.vector.memset(bias_t, -shift)

        rowpart = singles.tile([128, MT, NB], F32)
        rowsum_sb = singles.tile([128, MT], F32)
        diag_sb = singles.tile([128, MT], F32)

        # Remaining text blocks, round-robin across the three queues.
        nc.scalar.dma_start(bt_tiles[1], bt_d[1])
        nc.sync.dma_start(bt_tiles[2], bt_d[2])
        nc.gpsimd.dma_start(bt_tiles[3], bt_d[3])

        for nb in range(NB):
            colacc = caccp.tile([128, W], BF16, name=f"cacc{nb}", tag="cacc")
            for mt in range(MT):
                s_ps = psum.tile([128, W], F32, name=f"s{nb}_{mt}", tag="spsum")
                for kp in range(KP):
                    for h in range(NH):
                        nc.tensor.matmul(
                            s_ps[:, ts(h, MM_W)],
                            at_t[:, 2 * kp : 2 * kp + 2, ts(mt, 128)],
                            bt_tiles[nb][:, 2 * kp : 2 * kp + 2, ts(h, MM_W)],
                            start=(kp == 0),
                            stop=(kp == KP - 1),
                            perf_mode=mybir.MatmulPerfMode.DoubleRow,
                        )
                if nb == 0:
                    # this block holds the local diagonal block for every mt
                    o = mt * 128
                    dscr = scr.tile([128, 128], F32, name=f"dscr{mt}", tag="dscr")
                    nc.vector.tensor_mul(dscr, s_ps[:, o : o + 128], eye_t)
                    nc.vector.tensor_reduce(
                        out=diag_sb[:, mt : mt + 1],
                        in_=dscr,
                        axis=mybir.AxisListType.X,
                        op=mybir.AluOpType.add,
                    )
                e_t = expp.tile([128, W], BF16, name=f"e{nb}_{mt}", tag="exp")
                nc.scalar.activation(
                    e_t,
                    s_ps,
                    mybir.ActivationFunctionType.Exp,
                    bias=bias_t,
                    scale=act_scale,
                    accum_out=rowpart[:, mt, nb : nb + 1],
                )
                if mt == 0:
                    nc.vector.tensor_copy(colacc, e_t)
                else:
                    nc.vector.tensor_add(colacc, colacc, e_t)
            # Split the flush across SP + Pool queues (ACT queue is busy).
            nc.sync.dma_start(colsum_d[nb, :, 0:1024], colacc[:, 0:1024])
            nc.gpsimd.dma_start(colsum_d[nb, :, 1024:2048], colacc[:, 1024:2048])

        for mt in range(MT):
            nc.vector.tensor_reduce(
                out=rowsum_sb[:, mt : mt + 1],
                in_=rowpart[:, mt, :],
                axis=mybir.AxisListType.X,
                op=mybir.AluOpType.add,
            )
        nc.sync.dma_start(rowsum_d, rowsum_sb)
        nc.sync.dma_start(diag_d, diag_sb)

    nc.compile()
    return nc


def _prep_inputs(img, txt, scale):
    act_scale = scale / (FEAT_SCALE * FEAT_SCALE)
    eye = (act_scale * np.eye(128)).astype(np.float32)
    imgs = (img * FEAT_SCALE).astype(NP_FP8)
    txts = (txt * FEAT_SCALE).astype(NP_FP8)
    in_maps = []
    for c in range(NC):
        A = imgs[c * M_LOC : (c + 1) * M_LOC]                   # [1024, 512]
        at = np.ascontiguousarray(
            A.T.reshape(KC, 128, M_LOC).transpose(1, 0, 2)
        )                                                       # [128, 4, 1024]
        tr = np.roll(txts, -c * M_LOC, axis=0)                  # local col j -> global (j + c*1024) % N
        bt = np.ascontiguousarray(
            tr.T.reshape(KC, 128, NB, W).transpose(2, 1, 0, 3)
        )                                                       # [NB, 128, 4, W]
        in_maps.append({"at_in": at, "bt_in": bt, "eye_in": eye})
    return in_maps


def kernel(image_features, text_features, logit_scale):
    global LAST_RESULTS
    img = np.ascontiguousarray(np.asarray(image_features, dtype=np.float32))
    txt = np.ascontiguousarray(np.asarray(text_features, dtype=np.float32))
    scale = float(np.asarray(logit_scale))
    shift = 0.5 * scale

    key = (scale,)
    if key not in _CACHE:
        _CACHE[key] = _build(scale, shift)
    nc = _CACHE[key]

    in_maps = _prep_inputs(img, txt, scale)
    res = run_bass_kernel_spmd(nc, in_maps, core_ids=list(range(NC)))
    LAST_RESULTS = res

    colsum_tot = np.zeros(N, dtype=np.float64)
    lse_rows = []
    diags = []
    for c, r in enumerate(res.results):
        rowsum = r["rowsum_out"].astype(np.float64)             # [128, MT] @ [p, mt]
        lse_rows.append(shift + np.log(rowsum.T.reshape(-1)))   # row = mt*128 + p
        diags.append(r["diag_out"].astype(np.float64).T.reshape(-1))
        colsum_tot += np.roll(
            r["colsum_out"].astype(np.float64).sum(axis=1).reshape(-1), c * M_LOC
        )
    lse_row = np.concatenate(lse_rows)
    diag = np.concatenate(diags)
    lse_col = shift + np.log(colsum_tot)

    loss = 0.5 * (np.mean(lse_row - diag) + np.mean(lse_col - diag))
    return np.float32(loss)
